# revision 1
# baseline (speedup 1.0000x reference)
"""Trainium2 Bass kernel for nn_DiagonalVariance: per-dim MLPs [4->64->64->1] with softplus.

Strategy (pure data parallel over batch, 8 cores):
  - Host packs x^T = [y^T; t^T; ones] as [20, B] so all device DMAs are contiguous.
  - Per dim-pair p (2 dims), weights are packed as:
      W1p [20, 128]  (y-rows are delta-masked per dim, t-rows shared, last row = b1)
      W2p [128, 128] block-diagonal of two 64x64 blocks
      W3p [128, 32]  cols 0/1 hold W3 for the two dims, rest zero
  - softplus(z) = Ln(Exp(z) + 1): two ACT passes; the activation table set is
    pinned to natural_log_exp_and_others so it loads exactly once. The kernel
    is bound by ScalarE throughput (1 elem/cycle/lane for every ACT func):
    ~2 passes over 33.5M hidden elements per core. b2/b3 are fused into the
    Exp pass via per-partition bias APs; b1 rides a ones-row of x^T.
  - E=exp(z) is stored as fp16 (rounding E perturbs softplus by <= relerr(E),
    and |z|<=~8 here so no overflow); Ln ops are merged across pair groups
    (ln_group) to amortize the ~550-cycle per-op ACT overhead.
  - Matmuls run as float32r (full-rate PE mode on fp32 data; plain fp32 is
    4 cycles/row). L3 accumulates all 8 pairs into one psum tile whose rows
    0..15 are the 16 output dims.
  - Output is written as [16, BC] per core and transposed on the host.
"""

import numpy as np
from contextlib import ExitStack, nullcontext

import concourse.bass as bass
import concourse.bacc as bacc
import concourse.tile as tile
from concourse import mybir
from concourse.hw_specs import get_activation_tables

F = mybir.ActivationFunctionType
FP32 = mybir.dt.float32
FP32R = mybir.dt.float32r
FP16 = mybir.dt.float16

B = 262144
D = 16
TE = 3
H = 64
NCORES = 8
BC = B // NCORES          # 32768 batch points per core
NB = 2048                 # batch tile per pair-step
NMM = 512                 # max fp32 moving free dim per matmul
NPAIR = D // 2            # 8 dim-pairs
NTILES = BC // NB

_ACT_SET = "natural_log_exp_and_others"


def _pin_act_tables(arch):
    """Restrict Exp/Ln to one table set so bacc emits a single table load."""
    tables = get_activation_tables(arch)
    for name, funcs in tables.items():
        if name != _ACT_SET:
            funcs.discard(F.Exp)
            funcs.discard(F.Ln)


def build(ntiles=NTILES, reps=1, mm_dtype=FP32R, fp16_e=True, nb=None, dve_copy=False, z_bufs=1, ln_group=2):
    nc = bacc.Bacc("TRN2", target_bir_lowering=False, debug=False,
                   enable_asserts=True, num_devices=NCORES)
    _pin_act_tables(nc.m.arch)
    NB = nb or globals()["NB"]
    E_DT = FP16 if fp16_e else mm_dtype

    xT = nc.dram_tensor("xT", [20, BC], mm_dtype, kind="ExternalInput")
    w1 = nc.dram_tensor("w1", [20, NPAIR * 128], mm_dtype, kind="ExternalInput")
    w2 = nc.dram_tensor("w2", [128, NPAIR * 128], mm_dtype, kind="ExternalInput")
    w3 = nc.dram_tensor("w3", [128, NPAIR * 128], mm_dtype, kind="ExternalInput")
    b2 = nc.dram_tensor("b2", [128, NPAIR], FP32, kind="ExternalInput")
    b3 = nc.dram_tensor("b3", [128, 1], FP32, kind="ExternalInput")
    # output row d holds dim d, contiguous batch columns
    out = nc.dram_tensor("out", [D, ntiles * NB], FP32, kind="ExternalOutput")

    def mm(out_ap, lhsT, rhs, **kw):
        nc.tensor.matmul(out_ap, lhsT, rhs, **kw)

    with tile.TileContext(nc) as tc:
        with ExitStack() as ctx:
            wpool = ctx.enter_context(tc.tile_pool(name="w", bufs=1))
            xpool = ctx.enter_context(tc.tile_pool(name="x", bufs=2))
            hpool1 = ctx.enter_context(tc.tile_pool(name="h1", bufs=2))
            hpool2 = ctx.enter_context(tc.tile_pool(name="h2", bufs=NPAIR // ln_group + 1))
            opool = ctx.enter_context(tc.tile_pool(name="o", bufs=2))
            epool = ctx.enter_context(tc.tile_pool(name="e", bufs=3))
            zpool1 = ctx.enter_context(tc.tile_pool(name="z1", bufs=z_bufs, space="PSUM"))
            zpool2 = ctx.enter_context(tc.tile_pool(name="z2", bufs=z_bufs, space="PSUM"))
            zpool3 = zpool1

            w1sb = wpool.tile([20, NPAIR * 128], mm_dtype)
            w2sb = wpool.tile([128, NPAIR * 128], mm_dtype)
            w3sb = wpool.tile([128, NPAIR * 128], mm_dtype)
            b2sb = wpool.tile([128, NPAIR], FP32)
            b3sb = wpool.tile([128, 1], FP32)
            nc.sync.dma_start(out=w1sb, in_=w1[:, :])
            nc.sync.dma_start(out=w2sb, in_=w2[:, :])
            nc.sync.dma_start(out=w3sb, in_=w3[:, :])
            nc.sync.dma_start(out=b2sb, in_=b2[:, :])
            nc.sync.dma_start(out=b3sb, in_=b3[:, :])

            loop_cm = tc.For_i(0, reps, 1) if reps > 1 else nullcontext()
            with loop_cm:
                for i in range(ntiles):
                    xt = xpool.tile([20, NB], mm_dtype)
                    nc.sync.dma_start(out=xt, in_=xT[:, i * NB:(i + 1) * NB])

                    G = ln_group
                    h2s = []
                    for g in range(NPAIR // G):
                        pg = range(g * G, (g + 1) * G)
                        e1g = epool.tile([128, G, NB], E_DT, tag="e")
                        for j, p in enumerate(pg):
                            z1 = zpool1.tile([128, NB], FP32, tag="z1")
                            for m in range(NB // NMM):
                                s = slice(m * NMM, (m + 1) * NMM)
                                mm(z1[:, s], w1sb[:, p * 128:(p + 1) * 128], xt[:, s],
                                   start=True, stop=True)
                            nc.scalar.activation(e1g[:, j, :], z1, F.Exp)
                        h1g = hpool1.tile([128, G, NB], mm_dtype)
                        nc.scalar.activation(h1g, e1g, F.Ln, bias=1.0)

                        e2g = epool.tile([128, G, NB], E_DT, tag="e")
                        for j, p in enumerate(pg):
                            z2 = zpool2.tile([128, NB], FP32)
                            for m in range(NB // NMM):
                                s = slice(m * NMM, (m + 1) * NMM)
                                mm(z2[:, s], w2sb[:, p * 128:(p + 1) * 128],
                                   h1g[:, j, s], start=True, stop=True)
                            nc.scalar.activation(e2g[:, j, :], z2, F.Exp,
                                                 bias=b2sb[:, p:p + 1])
                        h2g = hpool2.tile([128, G, NB], mm_dtype)
                        nc.scalar.activation(h2g, e2g, F.Ln, bias=1.0)
                        for j, p in enumerate(pg):
                            h2s.append(h2g[:, j, :])

                    # all 8 pairs accumulate into one [128, NB] psum tile;
                    # pair p's lhsT has its W3 columns at 2p/2p+1, so rows
                    # 0..15 collect all dims and rows 16..127 stay zero.
                    z3 = zpool3.tile([128, NB], FP32, tag="z1")
                    for m in range(NB // NMM):
                        s = slice(m * NMM, (m + 1) * NMM)
                        for p in range(NPAIR):
                            mm(z3[:, s], w3sb[:, p * 128:(p + 1) * 128],
                               h2s[p][:, s], start=(p == 0), stop=(p == NPAIR - 1))
                    o3 = opool.tile([D, NB], FP32)
                    if fp16_e:
                        e3 = epool.tile([128, NB], E_DT, tag="e")
                        nc.scalar.activation(e3[:D, :], z3[:D, :], F.Exp, bias=b3sb[:D, :])
                        nc.scalar.activation(o3, e3[:D, :], F.Ln, bias=1.0)
                    else:
                        nc.scalar.activation(o3, z3[:D, :], F.Exp, bias=b3sb[:D, :])
                        nc.scalar.activation(o3, o3, F.Ln, bias=1.0)
                    nc.sync.dma_start(out=out[:, i * NB:(i + 1) * NB], in_=o3)
    nc.compile()
    return nc


def _pack_inputs(t, y, W1, b1, W2, b2, W3, b3):
    """Host-side packing. Returns per-core input maps."""
    t = np.asarray(t, np.float32)
    y = np.asarray(y, np.float32)
    W1 = np.asarray(W1, np.float32)
    b1 = np.asarray(b1, np.float32)
    W2 = np.asarray(W2, np.float32)
    b2 = np.asarray(b2, np.float32)
    W3 = np.asarray(W3, np.float32)
    b3 = np.asarray(b3, np.float32)

    xT = np.empty((20, B), np.float32)
    xT[:D] = y.T
    xT[D:D + TE] = t.T
    xT[D + TE] = 1.0

    w1p = np.zeros((20, NPAIR * 128), np.float32)
    w2p = np.zeros((128, NPAIR * 128), np.float32)
    w3p = np.zeros((128, NPAIR * 128), np.float32)
    b2p = np.zeros((128, NPAIR), np.float32)
    b3p = np.zeros((128, 1), np.float32)
    for p in range(NPAIR):
        for a in range(2):
            d = 2 * p + a
            c = slice(p * 128 + 64 * a, p * 128 + 64 * a + 64)
            w1p[d, c] = W1[d, 0, :]
            w1p[D:D + TE, c] = W1[d, 1:1 + TE, :]
            w1p[D + TE, c] = b1[d, :]
            w2p[64 * a:64 * a + 64, p * 128 + 64 * a:p * 128 + 64 * a + 64] = W2[d]
            w3p[64 * a:64 * a + 64, p * 128 + d] = W3[d, :, 0]
            b2p[64 * a:64 * a + 64, p] = b2[d]
            b3p[d, 0] = b3[d, 0]

    in_maps = []
    for c in range(NCORES):
        in_maps.append({
            "xT": np.ascontiguousarray(xT[:, c * BC:(c + 1) * BC]),
            "w1": w1p, "w2": w2p, "w3": w3p, "b2": b2p, "b3": b3p,
        })
    return in_maps


def _unpack_output(results):
    return np.concatenate([results[c]["out"].T for c in range(NCORES)], axis=0)


def make_runner(nc):
    """Build a reusable jitted SPMD callable for `nc` (axon PJRT path)."""
    import jax
    from jax.sharding import Mesh, PartitionSpec, NamedSharding
    from jax.experimental.shard_map import shard_map
    from concourse import bass2jax

    bass2jax.install_neuronx_cc_hook()
    partition_name = nc.partition_id_tensor.name if nc.partition_id_tensor else None
    in_names, out_names, out_avals = [], [], []
    for alloc in nc.m.functions[0].allocations:
        if not isinstance(alloc, mybir.MemoryLocationSet):
            continue
        name = alloc.memorylocations[0].name
        if alloc.kind == "ExternalInput":
            if name != partition_name:
                in_names.append(name)
        elif alloc.kind == "ExternalOutput":
            out_names.append(name)
            out_avals.append(jax.core.ShapedArray(tuple(alloc.tensor_shape),
                                                  mybir.dt.np(alloc.dtype)))
    all_in = in_names + out_names + ([partition_name] if partition_name else [])

    def _body(*args):
        operands = list(args)
        if partition_name is not None:
            operands.append(bass2jax.partition_id_tensor())
        outs = bass2jax._bass_exec_p.bind(
            *operands, out_avals=tuple(out_avals),
            in_names=tuple(all_in), out_names=tuple(out_names),
            lowering_input_output_aliases=(), sim_require_finite=True,
            sim_require_nnan=True, nc=nc)
        return tuple(outs)

    mesh = Mesh(np.asarray(jax.devices()[:NCORES]), ("core",))
    n = len(in_names) + len(out_names)
    sharded = jax.jit(shard_map(_body, mesh=mesh,
                                in_specs=(PartitionSpec("core"),) * n,
                                out_specs=(PartitionSpec("core"),) * len(out_names),
                                check_rep=False), keep_unused=True)
    shard0 = NamedSharding(mesh, PartitionSpec("core"))
    zeros = [jax.device_put(np.zeros((NCORES * a.shape[0], *a.shape[1:]), a.dtype),
                            shard0) for a in out_avals]

    def stage(in_maps):
        return [jax.device_put(
            np.concatenate([np.asarray(in_maps[c][nm]) for c in range(NCORES)], axis=0),
            shard0) for nm in in_names]

    def run_staged(dev_in):
        out_arrs = sharded(*dev_in, *zeros)
        jax.block_until_ready(out_arrs)
        return out_arrs

    def run(in_maps):
        out_arrs = run_staged(stage(in_maps))
        return [
            {name: np.asarray(out_arrs[i]).reshape(NCORES, *out_avals[i].shape)[c]
             for i, name in enumerate(out_names)}
            for c in range(NCORES)
        ]

    run.stage = stage
    run.run_staged = run_staged
    run.out_names = out_names
    run.out_avals = out_avals
    return run


_CACHED = {}


def _get_runner():
    if "runner" not in _CACHED:
        _CACHED["runner"] = make_runner(build())
    return _CACHED["runner"]


def kernel(t, y, W1, b1, W2, b2, W3, b3):
    run = _get_runner()
    in_maps = _pack_inputs(t, y, W1, b1, W2, b2, W3, b3)
    results = run(in_maps)
    return _unpack_output(results)



# revision 4
# speedup vs baseline: 2.1316x; 2.1316x over previous
"""Trainium2 Bass kernel for nn_DiagonalVariance: per-dim MLPs [4->64->64->1] with softplus.

Strategy (pure data parallel over batch, 8 cores):
  - Host packs x^T = [y^T; t^T; ones] as [20, B] so all device DMAs are contiguous.
  - Per dim-pair p (2 dims), weights are packed as:
      W1p [20, 128]   (y-rows delta-masked per dim, t-rows shared, last row = b1)
      W2p [128, 128]  block-diagonal of two 64x64 blocks (fp16)
      W3v [128, 4*128] per-pair: 4 column-shifted variants (fp16) so the 4
                      512-col batch chunks of a tile land on psum rows 16j+d;
                      z3 is a single [128,512] bank with 64 live rows.
  - softplus(z) = Ln(Exp(z) + 1). The Exp half is split between ScalarE
    (exact, bf16 out) and the DVE (one fused tensor_scalar op computing
    round(z*K16 + B16) into int16, whose bits ARE bf16 2^(z*log2e) up to
    mantissa linearization: the classic exp bit-hack, |ln err| <= 0.030).
    The Ln half always runs on ScalarE (exact), reading the mixed bf16 tile.
    dve1/dve2 = number of pairs per layer whose Exp goes to the DVE.
  - PSUM is a ring of 3 [128,1024] slots (6 banks) + double-buffered z3
    (2 banks). The ring decouples PE from ACT/DVE consumers; L3 matmuls are
    interleaved right after each pair group's Ln2 so the PE never has a long
    ACT-idle serialization block (and stays warm through the tile).
  - Ln is batched over ln_group pairs per op (FD = ln_group*2048) to
    amortize the ~222-cycle ScalarE op overhead.
  - Output is written as [64, ntiles*512] (rows = 16 dims x 4 chunks) and
    unpacked on the host.
"""

import numpy as np
from contextlib import ExitStack, nullcontext

import concourse.bass as bass
import concourse.bacc as bacc
import concourse.tile as tile
from concourse import mybir
from concourse.hw_specs import get_activation_tables

F = mybir.ActivationFunctionType
ALU = mybir.AluOpType
FP32 = mybir.dt.float32
FP32R = mybir.dt.float32r
FP16 = mybir.dt.float16
BF16 = mybir.dt.bfloat16
I16 = mybir.dt.int16

B = 262144
D = 16
TE = 3
H = 64
NCORES = 8
BC = B // NCORES          # 32768 batch points per core
NB = 2048                 # batch tile
NMM = 512                 # psum-bank-limited moving free dim per matmul
NPAIR = D // 2            # 8 dim-pairs
NTILES = BC // NB
NJ = NB // NMM            # 4 row-groups in the packed z3 bank

K16 = (1 << 7) / np.log(2.0)          # bit-hack exp scale for bf16 bits
B16 = 127.0 * (1 << 7) - 0.0430 * (1 << 7)  # centered bias (round-to-nearest)

_ACT_SET = "natural_log_exp_and_others"


def _pin_act_tables(arch):
    """Restrict Exp/Ln to one table set so bacc emits a single table load."""
    tables = get_activation_tables(arch)
    for name, funcs in tables.items():
        if name != _ACT_SET:
            funcs.discard(F.Exp)
            funcs.discard(F.Ln)


def build(ntiles=NTILES, reps=1, dve1=8, dve2=6, zring_bufs=3):
    nc = bacc.Bacc("TRN2", target_bir_lowering=False, debug=False,
                   enable_asserts=True, num_devices=NCORES)
    _pin_act_tables(nc.m.arch)
    G = 2                      # pairs per Ln group
    NCH = NB // 1024           # z ring chunks per pair (2)
    GPT = NPAIR // G           # groups per tile (4)
    NG = ntiles * GPT          # groups per pass
    # ScalarE-exact pairs (the rest use the DVE bit-exp), spread across groups
    order = [7, 3, 5, 1, 6, 2, 4, 0]
    sc1 = set(order[:NPAIR - dve1])
    sc2 = set(order[:NPAIR - dve2])

    xT = nc.dram_tensor("xT", [20, ntiles * NB], FP32R, kind="ExternalInput")
    w1 = nc.dram_tensor("w1", [20, NPAIR * 128], FP32R, kind="ExternalInput")
    w2 = nc.dram_tensor("w2", [128, NPAIR * 128], FP16, kind="ExternalInput")
    w3 = nc.dram_tensor("w3", [128, NPAIR * NJ * 128], FP16, kind="ExternalInput")
    b2s = nc.dram_tensor("b2s", [128, NPAIR], FP32, kind="ExternalInput")
    b2k = nc.dram_tensor("b2k", [128, NPAIR], FP32, kind="ExternalInput")
    b3s = nc.dram_tensor("b3s", [64, 1], FP32, kind="ExternalInput")
    # row 16*j + d holds dim d of batch chunk j; tile i owns cols i*512..
    out = nc.dram_tensor("out", [16 * NJ, ntiles * NMM], FP32, kind="ExternalOutput")

    with tile.TileContext(nc) as tc:
        with ExitStack() as ctx:
            wpool = ctx.enter_context(tc.tile_pool(name="w", bufs=1))
            xpool = ctx.enter_context(tc.tile_pool(name="x", bufs=2))
            epool = ctx.enter_context(tc.tile_pool(name="e", bufs=5))
            h1pool = ctx.enter_context(tc.tile_pool(name="h1", bufs=3))
            h2pool = ctx.enter_context(tc.tile_pool(name="h2", bufs=3))
            opool = ctx.enter_context(tc.tile_pool(name="o", bufs=2))
            zring = ctx.enter_context(tc.tile_pool(name="zr", bufs=zring_bufs, space="PSUM"))
            z3pool = ctx.enter_context(tc.tile_pool(name="z3", bufs=2, space="PSUM"))

            w1sb = wpool.tile([20, NPAIR * 128], FP32R)
            w2sb = wpool.tile([128, NPAIR * 128], FP16)
            w3sb = wpool.tile([128, NPAIR * NJ * 128], FP16)
            b2ssb = wpool.tile([128, NPAIR], FP32)
            b2ksb = wpool.tile([128, NPAIR], FP32)
            b3sb = wpool.tile([64, 1], FP32)
            nc.sync.dma_start(out=w1sb, in_=w1[:, :])
            nc.sync.dma_start(out=w2sb, in_=w2[:, :])
            nc.sync.dma_start(out=w3sb, in_=w3[:, :])
            nc.sync.dma_start(out=b2ssb, in_=b2s[:, :])
            nc.sync.dma_start(out=b2ksb, in_=b2k[:, :])
            nc.sync.dma_start(out=b3sb, in_=b3s[:, :])

            loop_cm = tc.For_i(0, reps, 1) if reps > 1 else nullcontext()
            with loop_cm:
                # Software-pipelined over half-groups k (2 pairs each), skewed
                # so every engine's queue head is dependency-free:
                #   iter k emits P2V2(k-1) | A2(k-2), P3(k-2) | P1V1(k+1) | A1(k)
                st = {}      # group k -> dict of live tiles
                xts = {}     # tile index -> xt tile
                z3s = {}     # tile index -> z3 psum tile

                def pairs_of(k):
                    g = k % GPT
                    return [G * g + j for j in range(G)]

                def P1V1(k):
                    T = k // GPT
                    if k % GPT == 0:
                        xt = xpool.tile([20, NB], FP32R, tag="xt", name="xt")
                        nc.sync.dma_start(out=xt, in_=xT[:, T * NB:(T + 1) * NB])
                        xts[T] = xt
                        xts.pop(T - 2, None)
                    xt = xts[T]
                    e1g = epool.tile([128, G, NB], BF16, tag="e", name="e1g")
                    st[k] = {"e1": e1g}
                    for jp, p in enumerate(pairs_of(k)):
                        for c in range(NCH):
                            zc = zring.tile([128, 1024], FP32, tag="z", name="zc")
                            for m in range(2):
                                so = slice(m * NMM, (m + 1) * NMM)
                                sx = slice(c * 1024 + m * NMM, c * 1024 + (m + 1) * NMM)
                                nc.tensor.matmul(zc[:, so], w1sb[:, p * 128:(p + 1) * 128],
                                                 xt[:, sx], start=True, stop=True)
                            dst = e1g[:, jp, c * 1024:(c + 1) * 1024]
                            if p in sc1:
                                nc.scalar.activation(dst, zc, F.Exp)
                            else:
                                nc.vector.tensor_scalar(dst.bitcast(I16), zc,
                                                        float(K16), float(B16),
                                                        ALU.mult, ALU.add)

                def A1(k):
                    h1g = h1pool.tile([128, G, NB], FP16, tag="h1", name="h1g")
                    nc.scalar.activation(h1g, st[k]["e1"], F.Ln, bias=1.0)
                    st[k]["h1"] = h1g
                    del st[k]["e1"]

                def P2V2(k):
                    h1g = st[k]["h1"]
                    e2g = epool.tile([128, G, NB], BF16, tag="e", name="e2g")
                    st[k]["e2"] = e2g
                    for jp, p in enumerate(pairs_of(k)):
                        for c in range(NCH):
                            zc = zring.tile([128, 1024], FP32, tag="z", name="zc")
                            for m in range(2):
                                so = slice(m * NMM, (m + 1) * NMM)
                                sx = slice(c * 1024 + m * NMM, c * 1024 + (m + 1) * NMM)
                                nc.tensor.matmul(zc[:, so], w2sb[:, p * 128:(p + 1) * 128],
                                                 h1g[:, jp, sx], start=True, stop=True)
                            dst = e2g[:, jp, c * 1024:(c + 1) * 1024]
                            if p in sc2:
                                nc.scalar.activation(dst, zc, F.Exp,
                                                     bias=b2ssb[:, p:p + 1])
                            else:
                                nc.vector.tensor_scalar(dst.bitcast(I16), zc,
                                                        float(K16), b2ksb[:, p:p + 1],
                                                        ALU.mult, ALU.add)
                    del st[k]["h1"]

                def A2(k):
                    h2g = h2pool.tile([128, G, NB], FP16, tag="h2", name="h2g")
                    nc.scalar.activation(h2g, st[k]["e2"], F.Ln, bias=1.0)
                    st[k]["h2"] = h2g
                    del st[k]["e2"]

                def P3(k):
                    T = k // GPT
                    if k % GPT == 0:
                        z3s[T] = z3pool.tile([128, NMM], FP32, tag="z3", name="z3")
                        z3s.pop(T - 2, None)
                    z3 = z3s[T]
                    h2g = st[k]["h2"]
                    for jp, p in enumerate(pairs_of(k)):
                        for j in range(NJ):
                            first = (k % GPT == 0 and jp == 0 and j == 0)
                            last = (k % GPT == GPT - 1 and jp == G - 1 and j == NJ - 1)
                            v = (p * NJ + j) * 128
                            nc.tensor.matmul(z3[:, :], w3sb[:, v:v + 128],
                                             h2g[:, jp, j * NMM:(j + 1) * NMM],
                                             start=first, stop=last)
                    del st[k]
                    if k % GPT == GPT - 1:
                        e3 = opool.tile([64, NMM], BF16, tag="e3", name="e3")
                        nc.scalar.activation(e3, z3[:64, :], F.Exp, bias=b3sb)
                        o3 = opool.tile([64, NMM], FP32, tag="o3", name="o3")
                        nc.scalar.activation(o3, e3, F.Ln, bias=1.0)
                        nc.sync.dma_start(out=out[:, T * NMM:(T + 1) * NMM], in_=o3)

                for k in range(-1, NG + 2):
                    if 0 <= k - 1 < NG:
                        P2V2(k - 1)
                    if 0 <= k - 2 < NG:
                        A2(k - 2)
                        P3(k - 2)
                    if 0 <= k + 1 < NG:
                        P1V1(k + 1)
                    if 0 <= k < NG:
                        A1(k)
    nc.compile()
    return nc


def _pack_inputs(t, y, W1, b1, W2, b2, W3, b3):
    """Host-side packing. Returns per-core input maps."""
    t = np.asarray(t, np.float32)
    y = np.asarray(y, np.float32)
    W1 = np.asarray(W1, np.float32)
    b1 = np.asarray(b1, np.float32)
    W2 = np.asarray(W2, np.float32)
    b2 = np.asarray(b2, np.float32)
    W3 = np.asarray(W3, np.float32)
    b3 = np.asarray(b3, np.float32)

    xT = np.empty((20, B), np.float32)
    xT[:D] = y.T
    xT[D:D + TE] = t.T
    xT[D + TE] = 1.0

    w1p = np.zeros((20, NPAIR * 128), np.float32)
    w2p = np.zeros((128, NPAIR * 128), np.float16)
    w3p = np.zeros((128, NPAIR * NJ * 128), np.float16)
    b2sp = np.zeros((128, NPAIR), np.float32)
    b3sp = np.zeros((64, 1), np.float32)
    for p in range(NPAIR):
        for a in range(2):
            d = 2 * p + a
            c = slice(p * 128 + 64 * a, p * 128 + 64 * a + 64)
            w1p[d, c] = W1[d, 0, :]
            w1p[D:D + TE, c] = W1[d, 1:1 + TE, :]
            w1p[D + TE, c] = b1[d, :]
            w2p[64 * a:64 * a + 64, p * 128 + 64 * a:p * 128 + 64 * a + 64] = W2[d]
            for j in range(NJ):
                w3p[64 * a:64 * a + 64, (p * NJ + j) * 128 + 16 * j + d] = W3[d, :, 0]
            b2sp[64 * a:64 * a + 64, p] = b2[d]
    for j in range(NJ):
        b3sp[16 * j:16 * j + D, 0] = b3[:, 0]
    b2kp = (b2sp * np.float32(K16) + np.float32(B16)).astype(np.float32)

    in_maps = []
    for c in range(NCORES):
        in_maps.append({
            "xT": np.ascontiguousarray(xT[:, c * BC:(c + 1) * BC]),
            "w1": w1p, "w2": w2p, "w3": w3p,
            "b2s": b2sp, "b2k": b2kp, "b3s": b3sp,
        })
    return in_maps


def _unpack_output(results):
    outs = []
    for c in range(NCORES):
        o = results[c]["out"]                      # [64, ntiles*512]
        nt = o.shape[1] // NMM
        o = o.reshape(NJ, D, nt, NMM).transpose(2, 0, 3, 1)  # [nt, NJ, 512, D]
        outs.append(o.reshape(nt * NB, D))
    return np.concatenate(outs, axis=0)


def make_runner(nc):
    """Build a reusable jitted SPMD callable for `nc` (axon PJRT path)."""
    import jax
    from jax.sharding import Mesh, PartitionSpec, NamedSharding
    from jax.experimental.shard_map import shard_map
    from concourse import bass2jax

    bass2jax.install_neuronx_cc_hook()
    partition_name = nc.partition_id_tensor.name if nc.partition_id_tensor else None
    in_names, out_names, out_avals = [], [], []
    for alloc in nc.m.functions[0].allocations:
        if not isinstance(alloc, mybir.MemoryLocationSet):
            continue
        name = alloc.memorylocations[0].name
        if alloc.kind == "ExternalInput":
            if name != partition_name:
                in_names.append(name)
        elif alloc.kind == "ExternalOutput":
            out_names.append(name)
            out_avals.append(jax.core.ShapedArray(tuple(alloc.tensor_shape),
                                                  mybir.dt.np(alloc.dtype)))
    all_in = in_names + out_names + ([partition_name] if partition_name else [])

    def _body(*args):
        operands = list(args)
        if partition_name is not None:
            operands.append(bass2jax.partition_id_tensor())
        outs = bass2jax._bass_exec_p.bind(
            *operands, out_avals=tuple(out_avals),
            in_names=tuple(all_in), out_names=tuple(out_names),
            lowering_input_output_aliases=(), sim_require_finite=True,
            sim_require_nnan=True, nc=nc)
        return tuple(outs)

    mesh = Mesh(np.asarray(jax.devices()[:NCORES]), ("core",))
    n = len(in_names) + len(out_names)
    sharded = jax.jit(shard_map(_body, mesh=mesh,
                                in_specs=(PartitionSpec("core"),) * n,
                                out_specs=(PartitionSpec("core"),) * len(out_names),
                                check_rep=False), keep_unused=True)
    shard0 = NamedSharding(mesh, PartitionSpec("core"))
    zeros = [jax.device_put(np.zeros((NCORES * a.shape[0], *a.shape[1:]), a.dtype),
                            shard0) for a in out_avals]

    def stage(in_maps):
        return [jax.device_put(
            np.concatenate([np.asarray(in_maps[c][nm]) for c in range(NCORES)], axis=0),
            shard0) for nm in in_names]

    def run_staged(dev_in):
        out_arrs = sharded(*dev_in, *zeros)
        jax.block_until_ready(out_arrs)
        return out_arrs

    def run(in_maps):
        out_arrs = run_staged(stage(in_maps))
        return [
            {name: np.asarray(out_arrs[i]).reshape(NCORES, *out_avals[i].shape)[c]
             for i, name in enumerate(out_names)}
            for c in range(NCORES)
        ]

    run.stage = stage
    run.run_staged = run_staged
    run.out_names = out_names
    run.out_avals = out_avals
    return run


_CACHED = {}


def _get_runner():
    if "runner" not in _CACHED:
        _CACHED["runner"] = make_runner(build())
    return _CACHED["runner"]


def kernel(t, y, W1, b1, W2, b2, W3, b3):
    run = _get_runner()
    in_maps = _pack_inputs(t, y, W1, b1, W2, b2, W3, b3)
    results = run(in_maps)
    return _unpack_output(results)
